# revision 20
# baseline (speedup 1.0000x reference)
"""Trainium2 Bass kernel for a dense attention block.

Reference computation (per batch b, head h):
    att = (q @ k^T) / sqrt(D) + att_mask          # [S, S]
    att = where(padding_mask[b], -inf, att)
    out = softmax(att, -1) @ v                    # [S, D]

Shapes: q,k,v [4, 16, 2048, 64] f32; att_mask [1,1,2048,2048] f32;
padding_mask [4, 2048, 2048] bool.  Output [4, 16, 2048, 64] f32.

Sharding over 8 cores: core c handles batch b=c//2, heads h in
[8*(c%2), 8*(c%2)+8).  Each core computes 8 full attention heads.

Device algorithm (per core), transposed-score formulation so that the
softmax reduction lands on the PE contraction axis.  Per (head, 1024-wide
q-block) "block", per 128-wide k-chunk j, with each chunk split into two
512-wide q halves (h0/h1) so the PSUM score tiles are one bank each and
four can be in flight:

  S^T_jh [128k, 512q] = K_j @ Q^T_h      (fp16 matmul into PSUM)
  h0: E = exp(S^T/8) on ACT; EW_h0 = E * Wm_j  (true exp path)
  h1: EW_h1 = bitcast_f16(u16(184.66*S^T + W16a_j))   (one DVE op)

The h1 path is the Schraudolph exp approximation done additively in the
fp16 exponent domain: u16(a*s + (a'*att + b - 31000*pad)) bit-cast to
fp16 equals exp(s/8 + att) within ~1.7% rms, with masked entries driven
negative so the saturating f32->u16 convert clamps them to +0.0 (and
legitimate tiny weights too, which is correct).  So each chunk's PSUM
first touch runs on BOTH the ACT and DVE engines concurrently, the h1
half needs no separate mask-multiply at all, and the two W forms (fp16
multiplicative for h0 columns, int16 additive for h1 columns) together
occupy the same 8 MB an fp16 [S,S] W would.

  O^T += V'_j^T @ EW_j    (fp16 matmul; V' carries a ones column so row
                           64 of O^T is the softmax denominator)

The h0 mask-multiplies are split DVE / Pool-gpsimd.  All DMA is issued
from the otherwise-idle SP sequencer via hardware DGE; the host
pre-casts q/k/v (+ att_mask) to fp16 and pre-lays-out v with a ones
column so no converting (software-DGE) gathers are needed.

Epilogue, two blocks behind (so its whole chain hides under chunk
work): ACT+DVE copy O^T to SBUF f16, PE-transposes back to [q, d] (f16,
1 cycle/row), DVE takes 1/Z and scales; the f16 store goes out in device
layout [p, t, d] and the host untiles and upcasts it.

TimelineSim (which matches fresh-device HW within ~1%): 285.7us for
prologue + one 8-head pass; steady-state per-pass 227us with all four
compute engines at 79-85% busy (PE-bound).  Measured HW rel err vs the
f32 reference: 1.4e-2 (gate 2e-2).  NOTE the device clocks down under
sustained back-to-back load (per-pass degrades to ~540us); test.py
therefore times in a quiesced state.
"""

import sys

if "/opt/trn_rl_repo" not in sys.path:
    sys.path.insert(0, "/opt/trn_rl_repo")

import numpy as np

import concourse.bass as bass
import concourse.tile as tile
from concourse import bacc, mybir
from concourse.bass import ts
from concourse.bass_utils import run_bass_kernel_spmd
from concourse.masks import make_identity

F32 = mybir.dt.float32
F16 = mybir.dt.float16
I16 = mybir.dt.int16
U16 = mybir.dt.uint16
U8 = mybir.dt.uint8

B, H, S, D = 4, 16, 2048, 64
N_CORES = 8
OUT_DTYPE = np.float16   # device output dtype (host casts back to f32)
HPC = H // 2          # heads per core
KC = 128              # k-chunk (PSUM partition dim of S^T)
NKC = S // KC         # 16 k-chunks
QB = 1024             # q-block
NQB = S // QB         # q-blocks per head
HB = 512              # half-block (one PSUM bank of scores)
NT = QB // 128        # 128-row tiles per q-block
SCALE = 1.0 / np.sqrt(D)

# Schraudolph fp16 exp in the integer domain:
#   bitcast_f16(u16(SCH_AS*s + SCH_AM*att + SCH_B)) ~= exp(SCALE*s + att)
SCH_AS = float(1024.0 / np.log(2.0) * SCALE)   # 184.664 (scores)
SCH_AM = float(1024.0 / np.log(2.0))           # 1477.32 (mask)
SCH_B = 15360.0 - 44.6                         # f16 bias minus minimax C
SCH_PAD = -31000.0                             # drives masked entries < 0

# h0 mask-multiply routing: these k-chunks' multiplies run on Pool
POOL_M = frozenset({0, 2, 3, 4, 5, 6, 8, 10, 12, 14, 15})
# these k-chunks run the h1 half on ACT (true exp + multiply) instead of
# the DVE Schraudolph path, balancing ACT vs DVE load and lowering the
# fraction of approximated weights to 12/32
ACT_FULL = frozenset({1, 4, 7, 10})


def build_program(n_heads=HPC, repeat=1, stage='full'):
    """Build the per-core Bass program (SPMD: identical on all 8 cores).

    repeat>1 re-runs the head loop (timing aid: the device-side cost of one
    pass equals the per-repeat time delta, independent of dispatch latency).
    """
    nc = bacc.Bacc("TRN2", target_bir_lowering=False, debug=False,
                   num_devices=N_CORES)

    qT = nc.declare_dram_parameter("qT", [HPC, D, S], F16, isOutput=False)
    kT = nc.declare_dram_parameter("kT", [HPC, D, S], F16, isOutput=False)
    # v with ones column, pre-tiled: va[h, p, c, d] = v'[h, c*128+p, d]
    va = nc.declare_dram_parameter("va", [HPC, KC, NKC, D + 1], F16,
                                   isOutput=False)
    attT = nc.declare_dram_parameter("attT", [S, S], F16, isOutput=False)
    padT = nc.declare_dram_parameter("padT", [S, S], U8, isOutput=False)
    # device-layout output: out[h, qb, p, t, d] = out[h, qb*QB + t*128 + p, d]
    out = nc.declare_dram_parameter("out", [HPC, NQB, KC, NT, D], F16,
                                    isOutput=True)

    with tile.TileContext(nc, num_cores=N_CORES) as tc:
        with (
            tc.tile_pool(name="singles", bufs=1) as singles,
            tc.tile_pool(name="wprep", bufs=2) as wprep,
            tc.tile_pool(name="heads", bufs=2) as heads,
            tc.tile_pool(name="chunks", bufs=3) as chunks,
            tc.tile_pool(name="outs", bufs=2) as outs,
            tc.tile_pool(name="ewp", bufs=2) as ewpool,
            tc.tile_pool(name="sp", bufs=4, space="PSUM") as sp_pool,
            tc.tile_pool(name="op", bufs=2, space="PSUM") as op_pool,
        ):
            # ---- constants ----
            ident = singles.tile([128, 128], F16, tag="ident")
            make_identity(nc, ident[:])

            # ---- W tables, SBUF resident, per k-chunk j:
            #   wm[j]  f16 = exp(att)*(1-pad): h0 columns (all q for ACT_FULL)
            #   wa[j]  i16 = AM*att + B - 31000*pad on h1 columns (others)
            wm = [singles.tile([128, NQB, QB if j in ACT_FULL else HB], F16,
                               name=f"wm{j}", tag=f"wm{j}")
                  for j in range(NKC)]
            wa = [None if j in ACT_FULL else
                  singles.tile([128, NQB, HB], I16, name=f"wa{j}",
                               tag=f"wa{j}")
                  for j in range(NKC)]

            def load_head(h):
                kt_h = heads.tile([D, S], F16, tag="kt")
                nc.sync.dma_start(kt_h[:], kT[h])
                qt_h = heads.tile([D, S], F16, tag="qt")
                nc.sync.dma_start(qt_h[:], qT[h])
                vp = heads.tile([128, NKC, D + 1], F16, tag="vp")
                nc.sync.dma_start(vp[:], va[h])
                return kt_h, qt_h, vp

            # first head's loads go out before the (much larger) W-table
            # loads so the PE can start immediately
            head0 = load_head(0) if n_heads else None

            for j in range(NKC):
                att_blk = wprep.tile([128, S], F16, tag="att_blk")
                nc.sync.dma_start(att_blk[:], attT[ts(j, 128), :])
                pad_blk = wprep.tile([128, S], U8, tag="pad_blk")
                nc.sync.dma_start(pad_blk[:], padT[ts(j, 128), :])
                att2 = att_blk[:].rearrange("p (q c) -> p q c", c=QB)
                pad2 = pad_blk[:].rearrange("p (q c) -> p q c", c=QB)
                # multiplicative W (h0 always; h1 too for ACT_FULL chunks)
                mw = QB if j in ACT_FULL else HB
                expat = wprep.tile([128, NQB, QB], F16, tag="expat")
                nc.scalar.activation(expat[:, :, 0:mw], att2[:, :, 0:mw],
                                     mybir.ActivationFunctionType.Exp)
                padf = wprep.tile([128, NQB, QB], F16, tag="padf")
                nc.gpsimd.tensor_scalar(padf[:, :, 0:mw], pad2[:, :, 0:mw],
                                        -1.0, 1.0,
                                        mybir.AluOpType.mult,
                                        mybir.AluOpType.add)
                nc.vector.tensor_mul(wm[j][:], expat[:, :, 0:mw],
                                     padf[:, :, 0:mw])
                if j in ACT_FULL:
                    continue
                # h1 additive (exponent-domain) W
                ta = wprep.tile([128, NQB, HB], I16, tag="ta")
                nc.vector.tensor_scalar(ta[:], att2[:, :, HB:QB],
                                        SCH_AM, SCH_B,
                                        mybir.AluOpType.mult,
                                        mybir.AluOpType.add)
                nc.vector.scalar_tensor_tensor(wa[j][:], pad2[:, :, HB:QB],
                                               SCH_PAD, ta[:],
                                               mybir.AluOpType.mult,
                                               mybir.AluOpType.add)

            # ---- main loop: 3-stage software pipeline over (head, q-block)
            # blocks.  Stage A (block i): QK matmuls -> first touch -> EW.
            # Stage B (block i-1): PV matmuls, interleaved chunk-by-chunk
            # with stage A.  Stage C (block i-2): epilogue.
            blocks = [(h_rep % n_heads, qb)
                      for h_rep in range(n_heads * repeat)
                      for qb in range(NQB)]
            prev = None   # (ew_blk, vp_tile, h, qb) of the previous block
            prev2 = None  # (o_ps, h, qb): PV output awaiting its epilogue
            kt_h = qt_h = vp_cur = None
            for i in range(len(blocks) + 2):
                cur = blocks[i] if i < len(blocks) else None
                if cur is not None:
                    h, qb = cur
                    if qb == 0:
                        # V'/K/Q are double-buffered (stage B of the last
                        # block of head h runs concurrently with h+1's loads)
                        kt_h, qt_h, vp_cur = \
                            head0 if i == 0 else load_head(h)
                    ew_blk = ewpool.tile([128, NKC, QB], F16, tag="ewb")
                o_ps = None
                if prev is not None and stage in ("pv", "full"):
                    o_ps = op_pool.tile([D + 1, QB], F32, tag="op")
                for j in range(NKC):
                    if cur is not None:
                        # h0: QK -> ACT exp -> (DVE|Pool) * wm
                        s0 = sp_pool.tile([128, HB], F32, tag="sp")
                        nc.tensor.matmul(
                            s0[:], lhsT=kt_h[:, ts(j, 128)],
                            rhs=qt_h[:, qb * QB: qb * QB + HB],
                            start=True, stop=True)
                        # h1: QK -> fused DVE Schraudolph+mask into EW
                        s1 = sp_pool.tile([128, HB], F32, tag="sp")
                        nc.tensor.matmul(
                            s1[:], lhsT=kt_h[:, ts(j, 128)],
                            rhs=qt_h[:, qb * QB + HB: qb * QB + QB],
                            start=True, stop=True)
                        if stage in ("exp", "mult", "pv", "full"):
                            e512 = chunks.tile([128, HB], F16, tag="e16")
                            nc.scalar.activation(
                                e512[:], s0[:],
                                mybir.ActivationFunctionType.Exp,
                                scale=float(SCALE))
                            if j in ACT_FULL:
                                e512b = chunks.tile([128, HB], F16,
                                                    tag="e16b")
                                nc.scalar.activation(
                                    e512b[:], s1[:],
                                    mybir.ActivationFunctionType.Exp,
                                    scale=float(SCALE))
                            else:
                                nc.vector.scalar_tensor_tensor(
                                    ew_blk[:, j, HB:QB].bitcast(U16), s1[:],
                                    SCH_AS, wa[j][:, qb, :],
                                    mybir.AluOpType.mult,
                                    mybir.AluOpType.add)
                        if stage in ("mult", "pv", "full"):
                            eng = nc.gpsimd if j in POOL_M else nc.vector
                            eng.tensor_mul(ew_blk[:, j, 0:HB], e512[:],
                                           wm[j][:, qb, 0:HB])
                            if j in ACT_FULL:
                                nc.vector.tensor_mul(
                                    ew_blk[:, j, HB:QB], e512b[:],
                                    wm[j][:, qb, HB:QB])
                    if o_ps is not None:
                        p_ew, p_vp, _, _ = prev
                        for m in range(QB // HB):
                            nc.tensor.matmul(o_ps[:, ts(m, HB)],
                                             lhsT=p_vp[:, j, :],
                                             rhs=p_ew[:, j, ts(m, HB)],
                                             start=(j == 0),
                                             stop=(j == NKC - 1))

                # ---- normalize + transpose + store, two blocks behind (the
                # PV output finished a full iteration ago, so this chain
                # hides entirely under the current block's chunk work)
                if prev2 is not None and stage == "full":
                    p2_ps, ph, pqb = prev2
                    # PSUM -> SBUF f16, split across ACT and DVE (gpsimd
                    # cannot touch PSUM); f16 keeps the PE transposes at 1
                    # cycle/row
                    o_sb = outs.tile([D + 1, QB], F16, tag="o_sb")
                    nc.scalar.activation(o_sb[:, 0:HB], p2_ps[:, 0:HB],
                                         mybir.ActivationFunctionType.Copy)
                    nc.vector.tensor_copy(o_sb[:, HB:QB], p2_ps[:, HB:QB])
                    # transpose scratch: two PSUM score slots (one bank each);
                    # otz pads each Z column to 2 f16 so every PSUM write
                    # stays 4-byte aligned
                    otf = sp_pool.tile([128, NT, D], F16, tag="sp")
                    otz = sp_pool.tile([128, NT, 2], F16, tag="sp")
                    for t in range(NT):
                        nc.tensor.transpose(otf[:, t, :],
                                            o_sb[0:D, ts(t, 128)],
                                            ident[0:D, 0:D])
                        nc.tensor.transpose(otz[:, t, 0:1],
                                            o_sb[D:D + 1, ts(t, 128)],
                                            ident[D:D + 1, D:D + 1])
                    rz = outs.tile([128, NT], F32, tag="rz")
                    nc.vector.reciprocal(rz[:], otz[:, :, 0])
                    o_st = outs.tile([128, NT, D], F16, tag="o_st")
                    nc.vector.tensor_mul(
                        o_st[:], otf[:],
                        rz[:].broadcast_to((128, NT, D)))
                    nc.sync.dma_start(out[ph, pqb], o_st[:])
                if o_ps is not None:
                    _, _, ph, pqb = prev
                    prev2 = (o_ps, ph, pqb)
                else:
                    prev2 = None
                prev = (ew_blk, vp_cur, h, qb) if cur is not None else None
    nc.finalize()
    return nc


_CACHED_NC = None


def _get_program():
    global _CACHED_NC
    if _CACHED_NC is None:
        _CACHED_NC = build_program()
    return _CACHED_NC


def shard_inputs(q, k, v, att_mask, padding_mask):
    """Host-side sharding + layout/dtype transforms (no math)."""
    attT = np.ascontiguousarray(att_mask[0, 0].T.astype(np.float16))
    padT = [np.ascontiguousarray(padding_mask[b].T).view(np.uint8)
            for b in range(B)]
    ones = np.ones((S, 1), dtype=np.float16)
    in_maps = []
    for c in range(N_CORES):
        b, hh = divmod(c, 2)
        h0 = hh * HPC
        qc = q[b, h0:h0 + HPC].astype(np.float16)
        kc = k[b, h0:h0 + HPC].astype(np.float16)
        vc = v[b, h0:h0 + HPC].astype(np.float16)
        # va[h, p, c, d] = v'[h, c*128+p, d] with a ones column at d=64
        vca = np.concatenate(
            [vc, np.broadcast_to(ones, (HPC, S, 1))], axis=2)
        vca = vca.reshape(HPC, NKC, KC, D + 1).transpose(0, 2, 1, 3)
        in_maps.append({
            "qT": np.ascontiguousarray(qc.transpose(0, 2, 1)),
            "kT": np.ascontiguousarray(kc.transpose(0, 2, 1)),
            "va": np.ascontiguousarray(vca),
            "attT": attT,
            "padT": padT[b],
        })
    return in_maps


def untile_output(arr):
    """[HPC, NQB, 128, NT, D] f16 device layout -> [HPC, S, D] f32."""
    return arr.astype(np.float32).transpose(0, 1, 3, 2, 4).reshape(HPC, S, D)


def unshard_output(results):
    out = np.empty((B, H, S, D), dtype=np.float32)
    for c in range(N_CORES):
        b, hh = divmod(c, 2)
        h0 = hh * HPC
        out[b, h0:h0 + HPC] = untile_output(results[c]["out"])
    return out


def kernel(q, k, v, att_mask, padding_mask):
    q = np.asarray(q, dtype=np.float32)
    k = np.asarray(k, dtype=np.float32)
    v = np.asarray(v, dtype=np.float32)
    att_mask = np.asarray(att_mask, dtype=np.float32)
    padding_mask = np.asarray(padding_mask)
    nc = _get_program()
    in_maps = shard_inputs(q, k, v, att_mask, padding_mask)
    res = run_bass_kernel_spmd(nc, in_maps, list(range(N_CORES)))
    return unshard_output(res.results)
